# revision 17
# baseline (speedup 1.0000x reference)
# kernel.py — distributed GraphSAGE-style GNN on 8 Trainium2 NeuronCores.
#
# Strategy (1D node partition, per the sharding hint):
#   * Nodes/features sharded contiguously across 8 cores (12500 each). Edges
#     are partitioned by destination node so segment-sum stays core-local.
#   * Each block all-gathers the bf16 node-feature table; per-edge source
#     features are then fetched with local `dma_gather` custom DMAs (the
#     memory-bound inner loop). dma_gather indices are int16, so the table is
#     addressed in 4 row-range segments of <=32768 rows.
#   * Per-graph pooled embeddings are combined with an all-reduce; the small
#     weight matrices are replicated.
#
# Per 128-dst-node tile:
#   feats[e,h]  <- dma_gather of node_table[src[e]] (grouped calls)
#   onehot[e,d] = (iota[d] == rel[e]) * inv_deg[e]        (one DVE op/chunk)
#   aggT[h,d]   = sum_chunks feats_c (x) onehot_c          (PE, PSUM accum)
#   conv[d,h']  = aggT.T @ w_nbr + nodeT.T @ w_root        (PE)
#   gconv[g,:] += onehot_g.T @ (conv + b)                  (PE graph pooling)
#   x = conv + b + node ; LN stats via bn_stats/bn_aggr; node' = relu(LN(x))
#   nodeT'      = PE transpose of bf16(node')
#
# Matmuls in bf16 w/ f32 accumulation: ~5e-3 absmax rel err vs f32 reference.

import os
import sys

for _p in ("/opt/trn_rl_repo", "/root/.axon_site/_ro/trn_rl_repo"):
    if os.path.isdir(_p) and _p not in sys.path:
        sys.path.append(_p)

import numpy as np

import concourse.bacc as bacc
import concourse.bass as bass
import concourse.mybir as mybir
import concourse.tile as tile
from concourse import library_config
from concourse.masks import make_identity
from contextlib import ExitStack

BF16 = mybir.dt.bfloat16
F32 = mybir.dt.float32
I32 = mybir.dt.int32
I16 = mybir.dt.int16
AOP = mybir.AluOpType
AFT = mybir.ActivationFunctionType

LN_EPS = 1e-5
NCORES = 8
NSEG = 4          # int16 index range segments of the gather table
GROUP = 14        # dst tiles per gather index block
GCAP = 4          # chunks (x128 idxs) per dma_gather call


def full_config():
    return dict(N=100000, E=600000, H=128, G=64, NB=3, SH=12500)


def derived(cfg):
    SH = cfg["SH"]
    T = (SH + 127) // 128
    LAST = SH - (T - 1) * 128
    SEGSZ = -(-cfg["N"] // NSEG)
    NG = -(-T // GROUP)
    return T, LAST, SEGSZ, NG


class Plan:
    pass


def make_plan(cfg, edge_index, batch_np, inv):
    """Chunk structure shared by program + per-core index arrays."""
    N, SH = cfg["N"], cfg["SH"]
    T, LAST, SEGSZ, NG = derived(cfg)
    src = np.asarray(edge_index[0]).astype(np.int64)
    dst = np.asarray(edge_index[1]).astype(np.int64)

    order = np.argsort(dst, kind="stable")
    srcs, dsts = src[order], dst[order]

    # per core / tile / segment edge lists
    edges = []  # [core][t][q] -> (src_arr, dstloc_arr)
    cnt = np.zeros((NCORES, T, NSEG), np.int64)
    for c in range(NCORES):
        lo = np.searchsorted(dsts, c * SH)
        hi = np.searchsorted(dsts, (c + 1) * SH)
        s_c = srcs[lo:hi]
        loc = dsts[lo:hi] - c * SH
        t_e = loc // 128
        q_e = s_c // SEGSZ
        per_t = []
        for t in range(T):
            m = t_e == t
            st, lt, qt = s_c[m], loc[m], q_e[m]
            per_q = []
            for q in range(NSEG):
                mq = qt == q
                per_q.append((st[mq], lt[mq]))
                cnt[c, t, q] = mq.sum()
            per_t.append(per_q)
        edges.append(per_t)

    plan = Plan()
    plan.K_tq = np.ceil(cnt.max(axis=0) / 128).astype(np.int64)  # [T, NSEG]
    plan.K_t = plan.K_tq.sum(axis=1)  # chunks per tile
    plan.KMAX = int(plan.K_t.max())
    # group structure: group g covers tiles [g*GROUP, min((g+1)*GROUP, T))
    plan.C_gq = np.zeros((NG, NSEG), np.int64)  # chunks per gather call
    for g in range(NG):
        tl = list(range(g * GROUP, min((g + 1) * GROUP, T)))
        for q in range(NSEG):
            plan.C_gq[g, q] = plan.K_tq[tl, q].sum()
    plan.CMAX_q = plan.C_gq.max(axis=0)  # [NSEG]
    plan.WTOT = int(plan.CMAX_q.sum()) * 8  # int16 idx cols per group
    plan.edges = edges
    plan.cnt = cnt
    return plan


def structure_key(cfg, plan):
    return (
        tuple(sorted(cfg.items())),
        plan.K_tq.tobytes(),
        plan.C_gq.tobytes(),
    )


def build_in_maps(cfg, plan, x, fc_w, fc_b, w_root, w_nbr, b, ln_g, ln_b, batch_np, inv):
    N, H, G, NB, SH = (cfg[k] for k in ("N", "H", "G", "NB", "SH"))
    T, LAST, SEGSZ, NG = derived(cfg)
    KMAX = plan.KMAX
    in_maps = []
    for c in range(NCORES):
        meta = np.zeros((T, 128, 2 * KMAX + 1), np.float32)
        meta[:, :, :KMAX] = -1.0  # rel pad
        gidx = np.zeros((NG, 128, plan.WTOT), np.int16)
        for t in range(T):
            g, j0 = t // GROUP, 0
            for q in range(NSEG):
                st, lt = plan.edges[c][t][q]
                k_tq = int(plan.K_tq[t, q])
                if k_tq == 0:
                    continue
                n = len(st)
                if n:
                    s = np.arange(n)
                    p, ch = s % 128, s // 128
                    j = j0 + ch  # chunk column within tile
                    meta[t, p, j] = (lt - t * 128).astype(np.float32)
                    meta[t, p, KMAX + j] = inv[lt + c * SH]
                    # gather idx list position within call (g, q)
                    base_off = int(plan.K_tq[g * GROUP : t, q].sum())
                    flat = (base_off + ch) * 128 + p  # in [0, C_gq*128)
                    col0 = int(plan.CMAX_q[:q].sum()) * 8
                    vals = (st - q * SEGSZ).astype(np.int16)
                    gidx[g, flat % 16, col0 + flat // 16] = vals
                j0 += k_tq
        # replicate idx block across the 8 Q7 cores (16-partition wrap x 8)
        for r in range(1, 8):
            gidx[:, 16 * r : 16 * (r + 1), :] = gidx[:, 0:16, :]
        relg = np.full((T * 128,), -1.0, np.float32)
        relg[:SH] = batch_np[c * SH : (c + 1) * SH].astype(np.float32)
        meta[:, :, 2 * KMAX] = relg.reshape(T, 128)

        xs = np.zeros((T * 128, H), np.float32)
        xs[:SH] = np.asarray(x)[c * SH : (c + 1) * SH]

        in_maps.append(
            {
                "x": xs,
                "fc_w": np.asarray(fc_w, np.float32),
                "fc_b": np.asarray(fc_b, np.float32).reshape(1, H),
                "w_root": np.asarray(w_root, np.float32),
                "w_nbr": np.asarray(w_nbr, np.float32),
                "bvec": np.asarray(b, np.float32),
                "ln_gv": np.asarray(ln_g, np.float32),
                "ln_bv": np.asarray(ln_b, np.float32),
                "meta": meta,
                "gidx": gidx,
            }
        )
    return in_maps


def preprocess(cfg, x, edge_index, batch, fc_w, fc_b, w_root, w_nbr, b, ln_g, ln_b):
    N = cfg["N"]
    dst = np.asarray(edge_index[1]).astype(np.int64)
    batch_np = np.asarray(batch).astype(np.int64)
    deg = np.bincount(dst, minlength=N)
    inv = (1.0 / np.maximum(deg, 1.0)).astype(np.float32)
    plan = make_plan(cfg, edge_index, batch_np, inv)
    in_maps = build_in_maps(
        cfg, plan, x, fc_w, fc_b, w_root, w_nbr, b, ln_g, ln_b, batch_np, inv
    )
    return plan, in_maps


# ----------------------------------------------------------------------------
# Device program
# ----------------------------------------------------------------------------
def build_program(cfg, plan):
    N, H, G, NB, SH = (cfg[k] for k in ("N", "H", "G", "NB", "SH"))
    T, LAST, SEGSZ, NG = derived(cfg)
    KMAX = plan.KMAX
    RG = [list(range(NCORES))]

    nc = bacc.Bacc(
        "TRN2",
        target_bir_lowering=False,
        debug=False,
        enable_asserts=False,
        num_devices=NCORES,
    )

    x_ap = nc.dram_tensor("x", [T * 128, H], F32, kind="ExternalInput").ap()
    fc_w_ap = nc.dram_tensor("fc_w", [H, H], F32, kind="ExternalInput").ap()
    fc_b_ap = nc.dram_tensor("fc_b", [1, H], F32, kind="ExternalInput").ap()
    w_root_ap = nc.dram_tensor("w_root", [NB, H, H], F32, kind="ExternalInput").ap()
    w_nbr_ap = nc.dram_tensor("w_nbr", [NB, H, H], F32, kind="ExternalInput").ap()
    bvec_ap = nc.dram_tensor("bvec", [NB, H], F32, kind="ExternalInput").ap()
    ln_gv_ap = nc.dram_tensor("ln_gv", [NB, H], F32, kind="ExternalInput").ap()
    ln_bv_ap = nc.dram_tensor("ln_bv", [NB, H], F32, kind="ExternalInput").ap()
    meta_ap = nc.dram_tensor("meta", [T, 128, 2 * KMAX + 1], F32, kind="ExternalInput").ap()
    gidx_ap = nc.dram_tensor("gidx", [NG, 128, plan.WTOT], I16, kind="ExternalInput").ap()

    node_out_ap = nc.dram_tensor("node_out", [SH, H], F32, kind="ExternalOutput").ap()
    graph_out_ap = nc.dram_tensor("graph_out", [G, H], F32, kind="ExternalOutput").ap()

    with tile.TileContext(nc) as tc, ExitStack() as ctx:
        const = ctx.enter_context(tc.tile_pool(name="const", bufs=1))
        sb = ctx.enter_context(tc.tile_pool(name="sb", bufs=2))
        sb3 = ctx.enter_context(tc.tile_pool(name="sb3", bufs=3))
        ps = ctx.enter_context(tc.tile_pool(name="ps", bufs=2, space="PSUM"))
        ps1 = ctx.enter_context(tc.tile_pool(name="ps1", bufs=1, space="PSUM"))
        dram = ctx.enter_context(tc.tile_pool(name="dram", bufs=1, space="DRAM"))


        # ---------------- constants ----------------
        ident_bf = const.tile([128, 128], BF16, name="ident_bf")
        make_identity(nc, ident_bf[:, :])

        iota_i = const.tile([128, 128], I32, name="iota_i")
        nc.gpsimd.iota(iota_i[:, :], pattern=[[1, 128]], base=0, channel_multiplier=0)
        iota_f = const.tile([128, 128], F32, name="iota_f")
        nc.vector.tensor_copy(iota_f[:, :], iota_i[:, :])

        iog_i = const.tile([128, G], I32, name="iog_i")
        nc.gpsimd.iota(iog_i[:, :], pattern=[[1, G]], base=0, channel_multiplier=0)
        iog_f = const.tile([128, G], F32, name="iog_f")
        nc.vector.tensor_copy(iog_f[:, :], iog_i[:, :])

        ones1 = const.tile([1, 128], F32, name="ones1")
        nc.vector.memset(ones1[:, :], 1.0)

        eps_t = const.tile([128, 1], F32, name="eps_t")
        nc.vector.memset(eps_t[:, :], LN_EPS)

        fc_w_b = const.tile([128, 128], BF16, name="fc_w_b")
        nc.gpsimd.dma_start(out=fc_w_b[:, :], in_=fc_w_ap[:, :])
        w_nbr_b = const.tile([128, NB * 128], BF16, name="w_nbr_b")
        w_root_b = const.tile([128, NB * 128], BF16, name="w_root_b")
        for i in range(NB):
            nc.gpsimd.dma_start(out=w_nbr_b[:, i * 128 : (i + 1) * 128], in_=w_nbr_ap[i])
            nc.gpsimd.dma_start(out=w_root_b[:, i * 128 : (i + 1) * 128], in_=w_root_ap[i])

        def bcast_vec(vec_ap, name):
            row = sb.tile([1, 128], F32, name=f"{name}_row", tag="bcrow")
            nc.sync.dma_start(out=row[:, :], in_=vec_ap)
            pb = ps.tile([128, 128], F32, name=f"{name}_ps", tag="convp")
            nc.tensor.matmul(pb[:, :], lhsT=ones1[:, :], rhs=row[:, :], start=True, stop=True)
            bc = const.tile([128, 128], F32, name=f"{name}_bc")
            nc.scalar.copy(bc[:, :], pb[:, :])
            return bc

        fcb_bcast = bcast_vec(fc_b_ap[:, :], "fcb")
        b_bc = [bcast_vec(bvec_ap[i : i + 1, :], f"b{i}") for i in range(NB)]
        g_bc = [bcast_vec(ln_gv_ap[i : i + 1, :], f"g{i}") for i in range(NB)]
        lb_bc = [bcast_vec(ln_bv_ap[i : i + 1, :], f"lb{i}") for i in range(NB)]

        node_f32 = const.tile([128, T * 128], F32, name="node_f32")
        nodeT_b = const.tile([128, T * 128], BF16, name="nodeT_b")
        graph_acc = const.tile([G, 128], F32, name="graph_acc")

        ag_in = dram.tile([SH, H], BF16, name="ag_in")
        tables = [
            dram.tile([N, H], BF16, name=f"table{i}", addr_space="Shared")
            for i in range(NB)
        ]
        ar_in = dram.tile([G, H], F32, name="ar_in")
        ar_outs = [
            dram.tile([G, H], F32, name=f"ar_out{i}", addr_space="Shared")
            for i in range(NB)
        ]

        def rows_of(t):
            return 128 if t < T - 1 else LAST

        # ---------------- FC input projection ----------------
        for t in range(T):
            ts = slice(t * 128, (t + 1) * 128)
            rows = rows_of(t)
            xt = sb.tile([128, 128], F32, tag="xt", name="xt")
            nc.sync.dma_start(out=xt[:, :], in_=x_ap[ts, :])
            xb = sb.tile([128, 128], BF16, tag="xb", name="xb")
            nc.vector.tensor_copy(xb[:, :], xt[:, :])
            xTp = ps.tile([128, 128], BF16, tag="ntp", name="xTp")
            nc.tensor.transpose(xTp[:, :], xb[:, :], ident_bf[:, :])
            xT = sb.tile([128, 128], BF16, tag="xT", name="xT")
            nc.scalar.copy(xT[:, :], xTp[:, :])
            npfc = ps.tile([128, 128], F32, tag="convp", name="npfc")
            nc.tensor.matmul(npfc[:, :], lhsT=xT[:, :], rhs=fc_w_b[:, :], start=True, stop=True)
            nslot = node_f32[:, ts]
            nc.vector.tensor_add(nslot, npfc[:, :], fcb_bcast[:, :])
            nb = sb.tile([128, 128], BF16, tag="nb", name="nbfc")
            nc.vector.tensor_copy(nb[:, :], nslot)
            nc.sync.dma_start(out=ag_in[t * 128 : t * 128 + rows, :], in_=nb[:rows, :])
            ntp = ps.tile([128, 128], BF16, tag="ntp", name="ntpfc")
            nc.tensor.transpose(ntp[:, :], nb[:, :], ident_bf[:, :])
            nc.scalar.copy(nodeT_b[:, ts], ntp[:, :])

        nc.gpsimd.collective_compute(
            "AllGather", AOP.bypass, replica_groups=RG,
            ins=[ag_in.opt()], outs=[tables[0].opt()],
        )

        # ---------------- message-passing blocks ----------------
        for i in range(NB):
            table = tables[i]
            wnb = w_nbr_b[:, i * 128 : (i + 1) * 128]
            wrt = w_root_b[:, i * 128 : (i + 1) * 128]
            gconv_ps = ps1.tile([G, 128], F32, tag="gconv", name=f"gconv{i}")

            for g in range(NG):
                tiles_g = list(range(g * GROUP, min((g + 1) * GROUP, T)))
                C_g = [int(plan.C_gq[g, q]) for q in range(NSEG)]
                CT = sum(C_g)
                # per-group gather of all segments into one feats buffer
                gix = sb.tile([128, plan.WTOT], I16, tag="gix", name="gix")
                nc.sync.dma_start(out=gix[:, :], in_=gidx_ap[g])
                feats = sb.tile([128, max(CT, 1), 128], BF16, tag="feats", name="feats")
                coff = 0
                for q in range(NSEG):
                    cq = C_g[q]
                    if cq == 0:
                        continue
                    col0 = int(plan.CMAX_q[:q].sum()) * 8
                    seg_rows = min(SEGSZ, N - q * SEGSZ)
                    for k in range(0, cq, GCAP):
                        cc = min(GCAP, cq - k)
                        nc.gpsimd.dma_gather(
                            out_ap=feats[:, coff + k : coff + k + cc, :],
                            in_ap=table[q * SEGSZ : q * SEGSZ + seg_rows, :],
                            idxs_ap=gix[:, col0 + k * 8 : col0 + (k + cc) * 8],
                            num_idxs=cc * 128,
                            num_idxs_reg=cc * 128,
                            elem_size=H,
                        )
                    coff += cq
                # chunk offset of (t, q) inside feats
                feat_off = {}
                coff = 0
                for q in range(NSEG):
                    for t in tiles_g:
                        feat_off[(t, q)] = coff
                        coff += int(plan.K_tq[t, q])

                for t in tiles_g:
                    ts = slice(t * 128, (t + 1) * 128)
                    rows = rows_of(t)
                    meta_t = sb.tile([128, 2 * KMAX + 1], F32, tag="meta", name="meta_t")
                    nc.sync.dma_start(out=meta_t[:, :], in_=meta_ap[t])

                    K_t = int(plan.K_t[t])
                    aggT = ps.tile([128, 128], F32, tag="aggT", name="aggT")
                    j = 0
                    for q in range(NSEG):
                        for cch in range(int(plan.K_tq[t, q])):
                            oh = sb3.tile([128, 128], BF16, tag="oh", name="oh")
                            nc.vector.tensor_scalar(
                                oh[:, :], iota_f[:, :],
                                meta_t[:, j : j + 1], meta_t[:, KMAX + j : KMAX + j + 1],
                                AOP.is_equal, AOP.mult,
                            )
                            nc.tensor.matmul(
                                aggT[:, :],
                                lhsT=feats[:, feat_off[(t, q)] + cch, :],
                                rhs=oh[:, :],
                                start=(j == 0),
                                stop=(j == K_t - 1),
                            )
                            j += 1
                    aggTs = sb.tile([128, 128], BF16, tag="aggTs", name="aggTs")
                    nc.scalar.copy(aggTs[:, :], aggT[:, :])

                    convp = ps.tile([128, 128], F32, tag="convp", name="convp")
                    nc.tensor.matmul(convp[:, :], lhsT=aggTs[:, :], rhs=wnb, start=True, stop=False)
                    nc.tensor.matmul(convp[:, :], lhsT=nodeT_b[:, ts], rhs=wrt, start=False, stop=True)

                    t1 = sb.tile([128, 128], F32, tag="t1", name="t1")
                    nc.vector.tensor_add(t1[:, :], convp[:, :], b_bc[i][:, :])
                    poolb = sb.tile([128, 128], BF16, tag="poolb", name="poolb")
                    nc.scalar.copy(poolb[:, :], t1[:, :])
                    og = sb.tile([128, G], BF16, tag="og", name="og")
                    nc.vector.tensor_scalar(
                        og[:, :], iog_f[:, :], meta_t[:, 2 * KMAX : 2 * KMAX + 1], None,
                        AOP.is_equal,
                    )
                    nc.tensor.matmul(
                        gconv_ps[:, :], lhsT=og[:, :], rhs=poolb[:, :],
                        start=(t == 0), stop=(t == T - 1), skip_group_check=True,
                    )

                    x_t = sb.tile([128, 128], F32, tag="x_t", name="x_t")
                    nc.vector.tensor_add(x_t[:, :], t1[:, :], node_f32[:, ts])
                    st6 = sb.tile([128, 6], F32, tag="st6", name="st6")
                    nc.vector.bn_stats(st6[:, :], x_t[:, :])
                    st2 = sb.tile([128, 2], F32, tag="st2", name="st2")
                    nc.vector.bn_aggr(st2[:, :], st6[:, :])
                    std = sb.tile([128, 1], F32, tag="std", name="std")
                    nc.scalar.activation(std[:, :], st2[:, 1:2], AFT.Sqrt, bias=eps_t[:, :])
                    rstd = sb.tile([128, 1], F32, tag="rstd", name="rstd")
                    nc.vector.reciprocal(rstd[:, :], std[:, :])
                    u = sb.tile([128, 128], F32, tag="u", name="u")
                    nc.vector.tensor_scalar(
                        u[:, :], x_t[:, :], st2[:, 0:1], rstd[:, :], AOP.subtract, AOP.mult
                    )
                    z = sb.tile([128, 128], F32, tag="z", name="z")
                    nc.vector.tensor_mul(z[:, :], u[:, :], g_bc[i][:, :])
                    z2 = sb.tile([128, 128], F32, tag="z2", name="z2")
                    nc.vector.tensor_add(z2[:, :], z[:, :], lb_bc[i][:, :])

                    if i < NB - 1:
                        nslot = node_f32[:, ts]
                        nc.scalar.activation(nslot, z2[:, :], AFT.Relu)
                        nb = sb.tile([128, 128], BF16, tag="nb", name="nb")
                        nc.vector.tensor_copy(nb[:, :], nslot)
                        nc.sync.dma_start(
                            out=ag_in[t * 128 : t * 128 + rows, :], in_=nb[:rows, :]
                        )
                        ntp = ps.tile([128, 128], BF16, tag="ntp", name="ntp")
                        nc.tensor.transpose(ntp[:, :], nb[:, :], ident_bf[:, :])
                        nc.scalar.copy(nodeT_b[:, ts], ntp[:, :])
                    else:
                        nout = sb.tile([128, 128], F32, tag="nout", name="nout")
                        nc.scalar.activation(nout[:, :], z2[:, :], AFT.Relu)
                        nc.sync.dma_start(
                            out=node_out_ap[t * 128 : t * 128 + rows, :], in_=nout[:rows, :]
                        )

            # ---- graph pooling: all-reduce + LN + relu (replicated)
            gcs = sb.tile([G, 128], F32, tag="gcs", name="gcs")
            nc.vector.tensor_copy(gcs[:, :], gconv_ps[:, :])
            nc.sync.dma_start(out=ar_in[:, :], in_=gcs[:, :])
            nc.gpsimd.collective_compute(
                "AllReduce", AOP.add, replica_groups=RG,
                ins=[ar_in.opt()], outs=[ar_outs[i].opt()],
            )
            gs = sb.tile([G, 128], F32, tag="gs", name="gs")
            nc.sync.dma_start(out=gs[:, :], in_=ar_outs[i][:, :])
            gin = sb.tile([G, 128], F32, tag="gin", name="gin")
            if i == 0:
                nc.vector.tensor_copy(gin[:, :], gs[:, :])
            else:
                nc.vector.tensor_add(gin[:, :], gs[:, :], graph_acc[:, :])
            gst6 = sb.tile([G, 6], F32, tag="gst6", name="gst6")
            nc.vector.bn_stats(gst6[:, :], gin[:, :])
            gst2 = sb.tile([G, 2], F32, tag="gst2", name="gst2")
            nc.vector.bn_aggr(gst2[:, :], gst6[:, :])
            gstd = sb.tile([G, 1], F32, tag="gstd", name="gstd")
            nc.scalar.activation(gstd[:, :], gst2[:, 1:2], AFT.Sqrt, bias=eps_t[:G, :])
            grstd = sb.tile([G, 1], F32, tag="grstd", name="grstd")
            nc.vector.reciprocal(grstd[:, :], gstd[:, :])
            gu = sb.tile([G, 128], F32, tag="gu", name="gu")
            nc.vector.tensor_scalar(
                gu[:, :], gin[:, :], gst2[:, 0:1], grstd[:, :], AOP.subtract, AOP.mult
            )
            gz = sb.tile([G, 128], F32, tag="gz", name="gz")
            nc.vector.tensor_mul(gz[:, :], gu[:, :], g_bc[i][:G, :])
            gz2 = sb.tile([G, 128], F32, tag="gz2", name="gz2")
            nc.vector.tensor_add(gz2[:, :], gz[:, :], lb_bc[i][:G, :])
            nc.scalar.activation(graph_acc[:, :], gz2[:, :], AFT.Relu)
            if i == NB - 1:
                nc.sync.dma_start(out=graph_out_ap[:, :], in_=graph_acc[:, :])

            if i < NB - 1:
                nc.gpsimd.collective_compute(
                    "AllGather", AOP.bypass, replica_groups=RG,
                    ins=[ag_in.opt()], outs=[tables[i + 1].opt()],
                )

    nc.compile()
    return nc


_CACHE = {}


def get_program(cfg, plan):
    key = structure_key(cfg, plan)
    if key not in _CACHE:
        _CACHE[key] = build_program(cfg, plan)
    return _CACHE[key]


def run_spmd(nc, in_maps, trace=False, trace_cores=None):
    from concourse import bass_utils

    return bass_utils.run_bass_kernel_spmd(
        nc, in_maps, core_ids=list(range(NCORES)), trace=trace, trace_cores=trace_cores
    )


def kernel(x, edge_index, batch, fc_w, fc_b, w_root, w_nbr, b, ln_g, ln_b, trace=False):
    cfg = full_config()
    plan, in_maps = preprocess(
        cfg, x, edge_index, batch, fc_w, fc_b, w_root, w_nbr, b, ln_g, ln_b
    )
    nc = get_program(cfg, plan)
    res = run_spmd(nc, in_maps, trace=trace)
    kernel.last_result = res
    node = np.concatenate([r["node_out"] for r in res.results], axis=0)
    graph = res.results[0]["graph_out"]
    return node.astype(np.float32), graph.astype(np.float32)


# revision 18
# speedup vs baseline: 1.0760x; 1.0760x over previous
# kernel.py — distributed GraphSAGE-style GNN on 8 Trainium2 NeuronCores.
#
# Strategy (1D node partition, per the sharding hint):
#   * Nodes/features sharded contiguously across 8 cores (12500 each). Edges
#     are partitioned by destination node so segment-sum stays core-local.
#   * Each block all-gathers the bf16 node-feature table; per-edge source
#     features are then fetched with local `dma_gather` custom DMAs (the
#     memory-bound inner loop). dma_gather indices are int16, so the table is
#     addressed in 4 row-range segments of <=32768 rows.
#   * Per-graph pooled embeddings are combined with an all-reduce; the small
#     weight matrices are replicated.
#
# Per 128-dst-node tile:
#   feats[e,h]  <- dma_gather of node_table[src[e]] (grouped calls)
#   onehot[e,d] = (iota[d] == rel[e]) * inv_deg[e]        (one DVE op/chunk)
#   aggT[h,d]   = sum_chunks feats_c (x) onehot_c          (PE, PSUM accum)
#   conv[d,h']  = aggT.T @ w_nbr + nodeT.T @ w_root        (PE)
#   gconv[g,:] += onehot_g.T @ (conv + b)                  (PE graph pooling)
#   x = conv + b + node ; LN stats via bn_stats/bn_aggr; node' = relu(LN(x))
#   nodeT'      = PE transpose of bf16(node')
#
# Matmuls in bf16 w/ f32 accumulation: ~5e-3 absmax rel err vs f32 reference.

import os
import sys

for _p in ("/opt/trn_rl_repo", "/root/.axon_site/_ro/trn_rl_repo"):
    if os.path.isdir(_p) and _p not in sys.path:
        sys.path.append(_p)

import numpy as np

import concourse.bacc as bacc
import concourse.bass as bass
import concourse.mybir as mybir
import concourse.tile as tile
from concourse import library_config
from concourse.masks import make_identity
from contextlib import ExitStack

BF16 = mybir.dt.bfloat16
F32 = mybir.dt.float32
I32 = mybir.dt.int32
I16 = mybir.dt.int16
AOP = mybir.AluOpType
AFT = mybir.ActivationFunctionType

LN_EPS = 1e-5
NCORES = 8
NSEG = 4          # int16 index range segments of the gather table
GROUP = 14        # dst tiles per gather index block
GCAP = 8          # chunks (x128 idxs) per dma_gather call


def full_config():
    return dict(N=100000, E=600000, H=128, G=64, NB=3, SH=12500)


def derived(cfg):
    SH = cfg["SH"]
    T = (SH + 127) // 128
    LAST = SH - (T - 1) * 128
    SEGSZ = -(-cfg["N"] // NSEG)
    NG = -(-T // GROUP)
    return T, LAST, SEGSZ, NG


class Plan:
    pass


def make_plan(cfg, edge_index, batch_np, inv):
    """Chunk structure shared by program + per-core index arrays."""
    N, SH = cfg["N"], cfg["SH"]
    T, LAST, SEGSZ, NG = derived(cfg)
    src = np.asarray(edge_index[0]).astype(np.int64)
    dst = np.asarray(edge_index[1]).astype(np.int64)

    order = np.argsort(dst, kind="stable")
    srcs, dsts = src[order], dst[order]

    # per core / tile / segment edge lists
    edges = []  # [core][t][q] -> (src_arr, dstloc_arr)
    cnt = np.zeros((NCORES, T, NSEG), np.int64)
    for c in range(NCORES):
        lo = np.searchsorted(dsts, c * SH)
        hi = np.searchsorted(dsts, (c + 1) * SH)
        s_c = srcs[lo:hi]
        loc = dsts[lo:hi] - c * SH
        t_e = loc // 128
        q_e = s_c // SEGSZ
        per_t = []
        for t in range(T):
            m = t_e == t
            st, lt, qt = s_c[m], loc[m], q_e[m]
            per_q = []
            for q in range(NSEG):
                mq = qt == q
                per_q.append((st[mq], lt[mq]))
                cnt[c, t, q] = mq.sum()
            per_t.append(per_q)
        edges.append(per_t)

    plan = Plan()
    plan.K_tq = np.ceil(cnt.max(axis=0) / 128).astype(np.int64)  # [T, NSEG]
    plan.K_t = plan.K_tq.sum(axis=1)  # chunks per tile
    plan.KMAX = int(plan.K_t.max())
    # group structure: group g covers tiles [g*GROUP, min((g+1)*GROUP, T))
    plan.C_gq = np.zeros((NG, NSEG), np.int64)  # chunks per gather call
    for g in range(NG):
        tl = list(range(g * GROUP, min((g + 1) * GROUP, T)))
        for q in range(NSEG):
            plan.C_gq[g, q] = plan.K_tq[tl, q].sum()
    plan.CMAX_q = plan.C_gq.max(axis=0)  # [NSEG]
    plan.WTOT = int(plan.CMAX_q.sum()) * 8  # int16 idx cols per group
    plan.edges = edges
    plan.cnt = cnt
    return plan


def structure_key(cfg, plan):
    return (
        tuple(sorted(cfg.items())),
        plan.K_tq.tobytes(),
        plan.C_gq.tobytes(),
    )


def build_in_maps(cfg, plan, x, fc_w, fc_b, w_root, w_nbr, b, ln_g, ln_b, batch_np, inv):
    N, H, G, NB, SH = (cfg[k] for k in ("N", "H", "G", "NB", "SH"))
    T, LAST, SEGSZ, NG = derived(cfg)
    KMAX = plan.KMAX
    in_maps = []
    for c in range(NCORES):
        meta = np.zeros((T, 128, 2 * KMAX + 1), np.float32)
        meta[:, :, :KMAX] = -1.0  # rel pad
        gidx = np.zeros((NG, 128, plan.WTOT), np.int16)
        for t in range(T):
            g, j0 = t // GROUP, 0
            for q in range(NSEG):
                st, lt = plan.edges[c][t][q]
                k_tq = int(plan.K_tq[t, q])
                if k_tq == 0:
                    continue
                n = len(st)
                if n:
                    s = np.arange(n)
                    p, ch = s % 128, s // 128
                    j = j0 + ch  # chunk column within tile
                    meta[t, p, j] = (lt - t * 128).astype(np.float32)
                    meta[t, p, KMAX + j] = inv[lt + c * SH]
                    # gather idx list position within call (g, q)
                    base_off = int(plan.K_tq[g * GROUP : t, q].sum())
                    flat = (base_off + ch) * 128 + p  # in [0, C_gq*128)
                    col0 = int(plan.CMAX_q[:q].sum()) * 8
                    vals = (st - q * SEGSZ).astype(np.int16)
                    gidx[g, flat % 16, col0 + flat // 16] = vals
                j0 += k_tq
        # replicate idx block across the 8 Q7 cores (16-partition wrap x 8)
        for r in range(1, 8):
            gidx[:, 16 * r : 16 * (r + 1), :] = gidx[:, 0:16, :]
        relg = np.full((T * 128,), -1.0, np.float32)
        relg[:SH] = batch_np[c * SH : (c + 1) * SH].astype(np.float32)
        meta[:, :, 2 * KMAX] = relg.reshape(T, 128)

        xs = np.zeros((T * 128, H), np.float32)
        xs[:SH] = np.asarray(x)[c * SH : (c + 1) * SH]

        in_maps.append(
            {
                "x": xs,
                "fc_w": np.asarray(fc_w, np.float32),
                "fc_b": np.asarray(fc_b, np.float32).reshape(1, H),
                "w_root": np.asarray(w_root, np.float32),
                "w_nbr": np.asarray(w_nbr, np.float32),
                "bvec": np.asarray(b, np.float32),
                "ln_gv": np.asarray(ln_g, np.float32),
                "ln_bv": np.asarray(ln_b, np.float32),
                "meta": meta,
                "gidx": gidx,
            }
        )
    return in_maps


def preprocess(cfg, x, edge_index, batch, fc_w, fc_b, w_root, w_nbr, b, ln_g, ln_b):
    N = cfg["N"]
    dst = np.asarray(edge_index[1]).astype(np.int64)
    batch_np = np.asarray(batch).astype(np.int64)
    deg = np.bincount(dst, minlength=N)
    inv = (1.0 / np.maximum(deg, 1.0)).astype(np.float32)
    plan = make_plan(cfg, edge_index, batch_np, inv)
    in_maps = build_in_maps(
        cfg, plan, x, fc_w, fc_b, w_root, w_nbr, b, ln_g, ln_b, batch_np, inv
    )
    return plan, in_maps


# ----------------------------------------------------------------------------
# Device program
# ----------------------------------------------------------------------------
def build_program(cfg, plan):
    N, H, G, NB, SH = (cfg[k] for k in ("N", "H", "G", "NB", "SH"))
    T, LAST, SEGSZ, NG = derived(cfg)
    KMAX = plan.KMAX
    RG = [list(range(NCORES))]

    nc = bacc.Bacc(
        "TRN2",
        target_bir_lowering=False,
        debug=False,
        enable_asserts=False,
        num_devices=NCORES,
    )

    x_ap = nc.dram_tensor("x", [T * 128, H], F32, kind="ExternalInput").ap()
    fc_w_ap = nc.dram_tensor("fc_w", [H, H], F32, kind="ExternalInput").ap()
    fc_b_ap = nc.dram_tensor("fc_b", [1, H], F32, kind="ExternalInput").ap()
    w_root_ap = nc.dram_tensor("w_root", [NB, H, H], F32, kind="ExternalInput").ap()
    w_nbr_ap = nc.dram_tensor("w_nbr", [NB, H, H], F32, kind="ExternalInput").ap()
    bvec_ap = nc.dram_tensor("bvec", [NB, H], F32, kind="ExternalInput").ap()
    ln_gv_ap = nc.dram_tensor("ln_gv", [NB, H], F32, kind="ExternalInput").ap()
    ln_bv_ap = nc.dram_tensor("ln_bv", [NB, H], F32, kind="ExternalInput").ap()
    meta_ap = nc.dram_tensor("meta", [T, 128, 2 * KMAX + 1], F32, kind="ExternalInput").ap()
    gidx_ap = nc.dram_tensor("gidx", [NG, 128, plan.WTOT], I16, kind="ExternalInput").ap()

    node_out_ap = nc.dram_tensor("node_out", [SH, H], F32, kind="ExternalOutput").ap()
    graph_out_ap = nc.dram_tensor("graph_out", [G, H], F32, kind="ExternalOutput").ap()

    with tile.TileContext(nc) as tc, ExitStack() as ctx:
        const = ctx.enter_context(tc.tile_pool(name="const", bufs=1))
        sb = ctx.enter_context(tc.tile_pool(name="sb", bufs=2))
        sb3 = ctx.enter_context(tc.tile_pool(name="sb3", bufs=3))
        ps = ctx.enter_context(tc.tile_pool(name="ps", bufs=2, space="PSUM"))
        ps1 = ctx.enter_context(tc.tile_pool(name="ps1", bufs=1, space="PSUM"))
        dram = ctx.enter_context(tc.tile_pool(name="dram", bufs=1, space="DRAM"))


        # ---------------- constants ----------------
        ident_bf = const.tile([128, 128], BF16, name="ident_bf")
        make_identity(nc, ident_bf[:, :])

        iota_i = const.tile([128, 128], I32, name="iota_i")
        nc.gpsimd.iota(iota_i[:, :], pattern=[[1, 128]], base=0, channel_multiplier=0)
        iota_f = const.tile([128, 128], BF16, name="iota_f")
        nc.vector.tensor_copy(iota_f[:, :], iota_i[:, :])

        iog_i = const.tile([128, G], I32, name="iog_i")
        nc.gpsimd.iota(iog_i[:, :], pattern=[[1, G]], base=0, channel_multiplier=0)
        iog_f = const.tile([128, G], BF16, name="iog_f")
        nc.vector.tensor_copy(iog_f[:, :], iog_i[:, :])

        ones1 = const.tile([1, 128], F32, name="ones1")
        nc.vector.memset(ones1[:, :], 1.0)

        eps_t = const.tile([128, 1], F32, name="eps_t")
        nc.vector.memset(eps_t[:, :], LN_EPS)

        fc_w_b = const.tile([128, 128], BF16, name="fc_w_b")
        nc.gpsimd.dma_start(out=fc_w_b[:, :], in_=fc_w_ap[:, :])
        w_nbr_b = const.tile([128, NB * 128], BF16, name="w_nbr_b")
        w_root_b = const.tile([128, NB * 128], BF16, name="w_root_b")
        for i in range(NB):
            nc.gpsimd.dma_start(out=w_nbr_b[:, i * 128 : (i + 1) * 128], in_=w_nbr_ap[i])
            nc.gpsimd.dma_start(out=w_root_b[:, i * 128 : (i + 1) * 128], in_=w_root_ap[i])

        def bcast_vec(vec_ap, name):
            row = sb.tile([1, 128], F32, name=f"{name}_row", tag="bcrow")
            nc.sync.dma_start(out=row[:, :], in_=vec_ap)
            pb = ps.tile([128, 128], F32, name=f"{name}_ps", tag="convp")
            nc.tensor.matmul(pb[:, :], lhsT=ones1[:, :], rhs=row[:, :], start=True, stop=True)
            bc = const.tile([128, 128], F32, name=f"{name}_bc")
            nc.scalar.copy(bc[:, :], pb[:, :])
            return bc

        fcb_bcast = bcast_vec(fc_b_ap[:, :], "fcb")
        b_bc = [bcast_vec(bvec_ap[i : i + 1, :], f"b{i}") for i in range(NB)]
        g_bc = [bcast_vec(ln_gv_ap[i : i + 1, :], f"g{i}") for i in range(NB)]
        lb_bc = [bcast_vec(ln_bv_ap[i : i + 1, :], f"lb{i}") for i in range(NB)]

        node_f32 = const.tile([128, T * 128], F32, name="node_f32")
        nodeT_b = const.tile([128, T * 128], BF16, name="nodeT_b")
        graph_acc = const.tile([G, 128], F32, name="graph_acc")

        ag_in = dram.tile([SH, H], BF16, name="ag_in")
        tables = [
            dram.tile([N, H], BF16, name=f"table{i}", addr_space="Shared")
            for i in range(NB)
        ]
        ar_in = dram.tile([G, H], F32, name="ar_in")
        ar_outs = [
            dram.tile([G, H], F32, name=f"ar_out{i}", addr_space="Shared")
            for i in range(NB)
        ]

        def rows_of(t):
            return 128 if t < T - 1 else LAST

        # ---------------- FC input projection ----------------
        for t in range(T):
            ts = slice(t * 128, (t + 1) * 128)
            rows = rows_of(t)
            xt = sb.tile([128, 128], F32, tag="xt", name="xt")
            nc.sync.dma_start(out=xt[:, :], in_=x_ap[ts, :])
            xb = sb.tile([128, 128], BF16, tag="xb", name="xb")
            nc.vector.tensor_copy(xb[:, :], xt[:, :])
            xTp = ps.tile([128, 128], BF16, tag="ntp", name="xTp")
            nc.tensor.transpose(xTp[:, :], xb[:, :], ident_bf[:, :])
            xT = sb.tile([128, 128], BF16, tag="xT", name="xT")
            nc.scalar.copy(xT[:, :], xTp[:, :])
            npfc = ps.tile([128, 128], F32, tag="convp", name="npfc")
            nc.tensor.matmul(npfc[:, :], lhsT=xT[:, :], rhs=fc_w_b[:, :], start=True, stop=True)
            nslot = node_f32[:, ts]
            nc.vector.tensor_add(nslot, npfc[:, :], fcb_bcast[:, :])
            nb = sb.tile([128, 128], BF16, tag="nb", name="nbfc")
            nc.scalar.copy(nb[:, :], nslot)
            nc.sync.dma_start(out=ag_in[t * 128 : t * 128 + rows, :], in_=nb[:rows, :])
            ntp = ps.tile([128, 128], BF16, tag="ntp", name="ntpfc")
            nc.tensor.transpose(ntp[:, :], nb[:, :], ident_bf[:, :])
            nc.scalar.copy(nodeT_b[:, ts], ntp[:, :])

        nc.gpsimd.collective_compute(
            "AllGather", AOP.bypass, replica_groups=RG,
            ins=[ag_in.opt()], outs=[tables[0].opt()],
        )

        # ---------------- message-passing blocks ----------------
        for i in range(NB):
            table = tables[i]
            wnb = w_nbr_b[:, i * 128 : (i + 1) * 128]
            wrt = w_root_b[:, i * 128 : (i + 1) * 128]
            gconv_ps = ps1.tile([G, 128], F32, tag="gconv", name=f"gconv{i}")

            for g in range(NG):
                tiles_g = list(range(g * GROUP, min((g + 1) * GROUP, T)))
                C_g = [int(plan.C_gq[g, q]) for q in range(NSEG)]
                CT = sum(C_g)
                # per-group gather of all segments into one feats buffer
                gix = sb.tile([128, plan.WTOT], I16, tag="gix", name="gix")
                nc.sync.dma_start(out=gix[:, :], in_=gidx_ap[g])
                feats = sb.tile([128, max(CT, 1), 128], BF16, tag="feats", name="feats")
                coff = 0
                for q in range(NSEG):
                    cq = C_g[q]
                    if cq == 0:
                        continue
                    col0 = int(plan.CMAX_q[:q].sum()) * 8
                    seg_rows = min(SEGSZ, N - q * SEGSZ)
                    for k in range(0, cq, GCAP):
                        cc = min(GCAP, cq - k)
                        nc.gpsimd.dma_gather(
                            out_ap=feats[:, coff + k : coff + k + cc, :],
                            in_ap=table[q * SEGSZ : q * SEGSZ + seg_rows, :],
                            idxs_ap=gix[:, col0 + k * 8 : col0 + (k + cc) * 8],
                            num_idxs=cc * 128,
                            num_idxs_reg=cc * 128,
                            elem_size=H,
                        )
                    coff += cq
                # chunk offset of (t, q) inside feats
                feat_off = {}
                coff = 0
                for q in range(NSEG):
                    for t in tiles_g:
                        feat_off[(t, q)] = coff
                        coff += int(plan.K_tq[t, q])

                for t in tiles_g:
                    ts = slice(t * 128, (t + 1) * 128)
                    rows = rows_of(t)
                    meta_t = sb.tile([128, 2 * KMAX + 1], F32, tag="meta", name="meta_t")
                    nc.sync.dma_start(out=meta_t[:, :], in_=meta_ap[t])

                    K_t = int(plan.K_t[t])
                    aggT = ps.tile([128, 128], F32, tag="aggT", name="aggT")
                    j = 0
                    for q in range(NSEG):
                        for cch in range(int(plan.K_tq[t, q])):
                            oh = sb3.tile([128, 128], BF16, tag="oh", name="oh")
                            nc.vector.tensor_scalar(
                                oh[:, :], iota_f[:, :],
                                meta_t[:, j : j + 1], meta_t[:, KMAX + j : KMAX + j + 1],
                                AOP.is_equal, AOP.mult,
                            )
                            nc.tensor.matmul(
                                aggT[:, :],
                                lhsT=feats[:, feat_off[(t, q)] + cch, :],
                                rhs=oh[:, :],
                                start=(j == 0),
                                stop=(j == K_t - 1),
                            )
                            j += 1
                    aggTs = sb.tile([128, 128], BF16, tag="aggTs", name="aggTs")
                    nc.scalar.copy(aggTs[:, :], aggT[:, :])

                    convp = ps.tile([128, 128], F32, tag="convp", name="convp")
                    nc.tensor.matmul(convp[:, :], lhsT=aggTs[:, :], rhs=wnb, start=True, stop=False)
                    nc.tensor.matmul(convp[:, :], lhsT=nodeT_b[:, ts], rhs=wrt, start=False, stop=True)

                    t1 = sb.tile([128, 128], F32, tag="t1", name="t1")
                    nc.vector.tensor_add(t1[:, :], convp[:, :], b_bc[i][:, :])
                    poolb = sb.tile([128, 128], BF16, tag="poolb", name="poolb")
                    nc.scalar.copy(poolb[:, :], t1[:, :])
                    og = sb.tile([128, G], BF16, tag="og", name="og")
                    nc.vector.tensor_scalar(
                        og[:, :], iog_f[:, :], meta_t[:, 2 * KMAX : 2 * KMAX + 1], None,
                        AOP.is_equal,
                    )
                    nc.tensor.matmul(
                        gconv_ps[:, :], lhsT=og[:, :], rhs=poolb[:, :],
                        start=(t == 0), stop=(t == T - 1), skip_group_check=True,
                    )

                    x_t = sb.tile([128, 128], F32, tag="x_t", name="x_t")
                    nc.vector.tensor_add(x_t[:, :], t1[:, :], node_f32[:, ts])
                    st6 = sb.tile([128, 6], F32, tag="st6", name="st6")
                    nc.vector.bn_stats(st6[:, :], x_t[:, :])
                    st2 = sb.tile([128, 2], F32, tag="st2", name="st2")
                    nc.vector.bn_aggr(st2[:, :], st6[:, :])
                    std = sb.tile([128, 1], F32, tag="std", name="std")
                    nc.scalar.activation(std[:, :], st2[:, 1:2], AFT.Sqrt, bias=eps_t[:, :])
                    rstd = sb.tile([128, 1], F32, tag="rstd", name="rstd")
                    nc.vector.reciprocal(rstd[:, :], std[:, :])
                    u = sb.tile([128, 128], F32, tag="u", name="u")
                    nc.vector.tensor_scalar(
                        u[:, :], x_t[:, :], st2[:, 0:1], rstd[:, :], AOP.subtract, AOP.mult
                    )
                    z = sb.tile([128, 128], F32, tag="z", name="z")
                    nc.vector.tensor_mul(z[:, :], u[:, :], g_bc[i][:, :])
                    z2 = sb.tile([128, 128], F32, tag="z2", name="z2")
                    nc.vector.tensor_add(z2[:, :], z[:, :], lb_bc[i][:, :])

                    if i < NB - 1:
                        nslot = node_f32[:, ts]
                        nc.scalar.activation(nslot, z2[:, :], AFT.Relu)
                        nb = sb.tile([128, 128], BF16, tag="nb", name="nb")
                        nc.scalar.copy(nb[:, :], nslot)
                        nc.sync.dma_start(
                            out=ag_in[t * 128 : t * 128 + rows, :], in_=nb[:rows, :]
                        )
                        ntp = ps.tile([128, 128], BF16, tag="ntp", name="ntp")
                        nc.tensor.transpose(ntp[:, :], nb[:, :], ident_bf[:, :])
                        nc.scalar.copy(nodeT_b[:, ts], ntp[:, :])
                    else:
                        nout = sb.tile([128, 128], F32, tag="nout", name="nout")
                        nc.scalar.activation(nout[:, :], z2[:, :], AFT.Relu)
                        nc.sync.dma_start(
                            out=node_out_ap[t * 128 : t * 128 + rows, :], in_=nout[:rows, :]
                        )

            # ---- graph pooling: all-reduce + LN + relu (replicated)
            gcs = sb.tile([G, 128], F32, tag="gcs", name="gcs")
            nc.vector.tensor_copy(gcs[:, :], gconv_ps[:, :])
            nc.sync.dma_start(out=ar_in[:, :], in_=gcs[:, :])
            nc.gpsimd.collective_compute(
                "AllReduce", AOP.add, replica_groups=RG,
                ins=[ar_in.opt()], outs=[ar_outs[i].opt()],
            )
            gs = sb.tile([G, 128], F32, tag="gs", name="gs")
            nc.sync.dma_start(out=gs[:, :], in_=ar_outs[i][:, :])
            gin = sb.tile([G, 128], F32, tag="gin", name="gin")
            if i == 0:
                nc.vector.tensor_copy(gin[:, :], gs[:, :])
            else:
                nc.vector.tensor_add(gin[:, :], gs[:, :], graph_acc[:, :])
            gst6 = sb.tile([G, 6], F32, tag="gst6", name="gst6")
            nc.vector.bn_stats(gst6[:, :], gin[:, :])
            gst2 = sb.tile([G, 2], F32, tag="gst2", name="gst2")
            nc.vector.bn_aggr(gst2[:, :], gst6[:, :])
            gstd = sb.tile([G, 1], F32, tag="gstd", name="gstd")
            nc.scalar.activation(gstd[:, :], gst2[:, 1:2], AFT.Sqrt, bias=eps_t[:G, :])
            grstd = sb.tile([G, 1], F32, tag="grstd", name="grstd")
            nc.vector.reciprocal(grstd[:, :], gstd[:, :])
            gu = sb.tile([G, 128], F32, tag="gu", name="gu")
            nc.vector.tensor_scalar(
                gu[:, :], gin[:, :], gst2[:, 0:1], grstd[:, :], AOP.subtract, AOP.mult
            )
            gz = sb.tile([G, 128], F32, tag="gz", name="gz")
            nc.vector.tensor_mul(gz[:, :], gu[:, :], g_bc[i][:G, :])
            gz2 = sb.tile([G, 128], F32, tag="gz2", name="gz2")
            nc.vector.tensor_add(gz2[:, :], gz[:, :], lb_bc[i][:G, :])
            nc.scalar.activation(graph_acc[:, :], gz2[:, :], AFT.Relu)
            if i == NB - 1:
                nc.sync.dma_start(out=graph_out_ap[:, :], in_=graph_acc[:, :])

            if i < NB - 1:
                nc.gpsimd.collective_compute(
                    "AllGather", AOP.bypass, replica_groups=RG,
                    ins=[ag_in.opt()], outs=[tables[i + 1].opt()],
                )

    nc.compile()
    return nc


_CACHE = {}


def get_program(cfg, plan):
    key = structure_key(cfg, plan)
    if key not in _CACHE:
        _CACHE[key] = build_program(cfg, plan)
    return _CACHE[key]


def run_spmd(nc, in_maps, trace=False, trace_cores=None):
    from concourse import bass_utils

    return bass_utils.run_bass_kernel_spmd(
        nc, in_maps, core_ids=list(range(NCORES)), trace=trace, trace_cores=trace_cores
    )


def kernel(x, edge_index, batch, fc_w, fc_b, w_root, w_nbr, b, ln_g, ln_b, trace=False):
    cfg = full_config()
    plan, in_maps = preprocess(
        cfg, x, edge_index, batch, fc_w, fc_b, w_root, w_nbr, b, ln_g, ln_b
    )
    nc = get_program(cfg, plan)
    res = run_spmd(nc, in_maps, trace=trace)
    kernel.last_result = res
    node = np.concatenate([r["node_out"] for r in res.results], axis=0)
    graph = res.results[0]["graph_out"]
    return node.astype(np.float32), graph.astype(np.float32)
